# revision 19
# baseline (speedup 1.0000x reference)
"""DotInteraction Trainium2 kernel (int8-input version).

Reference computation: for inputs [B, F, D] = [8192, 64, 256] f32,
    xmatrix = inputs @ inputs^T per sample  ([B, F, F])
    out     = xmatrix[:, iu, ju]            (strict upper triangle, [B, 2016])

Strategy (pure data parallel over 8 NeuronCores, 1024 samples each):
  * Host quantizes rows to int8 with per-row scales s[b,f] = maxabs/127
    (rms rel err ~1e-2, under the 2e-2 gate with 2x margin) and ships
    int8 -> halves HBM-in bytes vs fp16 (the baseline's bottleneck: DMA
    was 99.8% busy at ~331 GB/s).
  * On-chip the int8 tiles are widened to fp16 (exact for |q|<=127):
    - CAST_CHUNKS ride the SWDGE cast-DMA path (nc.gpsimd.dma_start
      int8 HBM -> fp16 SBUF), spending spare SBUF-fabric bandwidth but
      no extra HBM bytes or engine time;
    - the rest are DMA'd as int8 (HWDGE/SP ring) and converted by DVE
      tensor-copies (ACT converts are slower and ACT is evac-loaded).
  * Matmuls unchanged from the fp16 kernel: per pair of samples the
    stationary operand is [K=128, M=128] (two samples side by side,
    FWL-eligible), moving operand the same AP; 2 k-blocks accumulate
    into one PSUM region; 4 pairs per 2KB PSUM bank, 2 banks per PSUM
    tile so evacuation runs as fewer, larger ops (fixed cost ~260ns/op).
  * Evacuation multiplies by 1/256 during the PSUM->SBUF fp16 copy
    (integer grams reach 163k > fp16 max; /256 keeps them in range and
    exactly representable), split mostly-ACT/some-DVE to balance load.
  * Output DMAs ride the SP HWDGE ring (Sync engine is otherwise idle;
    issuing them from ACT costs ~0.7us of ACT time each).
  * Host multiplies the gathered triangle by 256 * s_f * s_g.
"""

import os
import sys

import numpy as np

for _p in ("/opt/trn_rl_repo", "/root/.axon_site/_ro/trn_rl_repo"):
    if os.path.isdir(_p) and _p not in sys.path:
        sys.path.insert(0, _p)

import bass_rust  # noqa: E402
from concourse import bacc, bass, mybir, tile  # noqa: E402
from concourse.bass_utils import run_bass_kernel_spmd  # noqa: E402

B, F, D = 8192, 64, 256
N_CORES = 8
B_CORE = B // N_CORES            # 1024
TOT_PAIRS = B_CORE // 2          # 512 pairs per core
# Small first/last chunks shorten the pipeline ramp and drain tails.
CHUNK_PAIRS = [8, 24] + [32] * 14 + [16, 8, 8]
assert sum(CHUNK_PAIRS) == TOT_PAIRS
N_CHUNKS = len(CHUNK_PAIRS)
KB = 2                           # k-blocks of 128 over D

# chunks whose input rides the SWDGE cast-DMA path (int8 HBM -> fp16 SBUF)
CAST_CHUNKS = {6, 12}
# evac engine pattern per 2-bank PSUM tile (cycled): mostly ACT, some DVE
EVAC_PATTERN = ["s"] * 12 + ["v"]
# emit chunk i's output DMAs alongside chunk i+OUT_DEFER's inputs: the SP
# HWDGE ring is FIFO, so an out-DMA that still waits on its evac would
# head-of-line-block the in-DMAs queued behind it (cost ~34us of PE stall
# when outs were emitted eagerly).
OUT_DEFER = 6

I8 = mybir.dt.int8
FP16 = mybir.dt.float16
FP32 = mybir.dt.float32

EVAC_SCALE = 1.0 / 256.0

_cache = {}


def _dep(a, b, sync, reason):
    bass_rust.add_dep_helper(a.ins, b.ins, sync=sync, reason=reason)


def _build():
    nc = bacc.Bacc()
    # [kb, d, pair, half, f]  (pair-flat; chunks are pair ranges)
    xq = nc.declare_dram_parameter(
        "xq", [KB, 128, TOT_PAIRS, 2, F], I8, isOutput=False
    )
    # [half, f, pair, g]
    out = nc.declare_dram_parameter(
        "out", [2, F, TOT_PAIRS, F], FP16, isOutput=True
    )

    evac_i = 0

    with tile.TileContext(nc) as tc:
        with (
            tc.tile_pool(name="xq", bufs=7) as qpool,
            tc.tile_pool(name="xf", bufs=9) as xpool,
            tc.tile_pool(name="gram", bufs=7) as gpool,
            tc.tile_pool(name="ps", bufs=4, space=bass.MemorySpace.PSUM) as pspool,
        ):
            pending_outs = []
            p0 = 0
            for ci, npairs in enumerate(CHUNK_PAIRS):
                p1 = p0 + npairs
                while pending_outs and pending_outs[0][0] <= ci - OUT_DEFER:
                    _, g, q0, q1 = pending_outs.pop(0)
                    nc.sync.dma_start(
                        out=out[0, :, q0:q1, :], in_=g[0:64, 0, : q1 - q0, :]
                    )
                    nc.sync.dma_start(
                        out=out[1, :, q0:q1, :], in_=g[64:128, 1, : q1 - q0, :]
                    )
                xk = []
                for kb in range(KB):
                    xtile = xpool.tile([128, 32, 2, F], FP16, tag="xf")
                    if ci in CAST_CHUNKS:
                        nc.gpsimd.dma_start(
                            out=xtile[:, :npairs, :, :],
                            in_=xq[kb, :, p0:p1, :, :],
                        )
                    else:
                        qtile = qpool.tile([128, 32, 2, F], I8, tag="xq")
                        nc.sync.dma_start(
                            out=qtile[:, :npairs, :, :],
                            in_=xq[kb, :, p0:p1, :, :],
                        )
                        nc.vector.tensor_copy(
                            xtile[:, :npairs, :, :], qtile[:, :npairs, :, :]
                        )
                    xk.append(xtile)

                # [p, h, q, g]: h outermost so each partition's useful half
                # (h=0 for A-rows, h=1 for B-rows) is one contiguous run.
                gram = gpool.tile([128, 2, 32, F], FP16, tag="gram")

                for b in range(npairs // 8):
                    # One PSUM tile = 2 banks = 8 pairs.  Each bank is its
                    # own accumulation region: start=True zeroes the whole
                    # 2KB bank, so the first matmul of EACH bank (j=0 and
                    # j=4 at kb=0) must run before that bank's others.
                    ps = pspool.tile([128, 8, 2, F], FP32, tag="ps")
                    mms = []
                    for kb in range(KB):
                        for j in range(8):
                            q = 8 * b + j
                            s = xk[kb][:, q, :, :]   # [128, 2, 64]
                            mms.append(
                                nc.tensor.matmul(
                                    ps[:, j, :, :],
                                    s,
                                    s,
                                    start=(kb == 0 and j % 4 == 0),
                                    stop=(kb == KB - 1 and j % 4 == 3),
                                    skip_group_check=True,
                                )
                            )
                    # bank0 first-MM = mms[0] (kb0,j0); bank1 = mms[4] (kb0,j4)
                    for k, mm in enumerate(mms):
                        first = mms[0] if k % 8 < 4 else mms[4]
                        if mm is not first:
                            _dep(mm, first, False, "bank zero-region order")
                    # 2-bank PSUM->SBUF copy with the 1/256 dequant scale
                    # folded in; mostly ACT (DVE carries the converts).
                    # scheduler-assigned engine: adapts the DVE/ACT
                    # split to runtime load instead of a static pattern
                    nc.any.tensor_scalar_mul(
                        gram[:, :, 8 * b : 8 * b + 8, :],
                        ps[:].transpose([0, 2, 1, 3]),
                        EVAC_SCALE,
                    )

                # sample 2q   lives at partitions 0:64,   (h=0, q, :)
                # sample 2q+1 lives at partitions 64:128, (h=1, q, :)
                pending_outs.append((ci, gram, p0, p1))
                p0 = p1
            for _, g, q0, q1 in pending_outs:
                nc.sync.dma_start(
                    out=out[0, :, q0:q1, :], in_=g[0:64, 0, : q1 - q0, :]
                )
                nc.sync.dma_start(
                    out=out[1, :, q0:q1, :], in_=g[64:128, 1, : q1 - q0, :]
                )
    nc.compile()
    return nc


def _get_nc():
    if "nc" not in _cache:
        _cache["nc"] = _build()
    return _cache["nc"]


def _quantize(inputs):
    """Per-row int8 quantization. Returns (q [B,F,D] int8, s [B,F] f32)."""
    s = np.abs(inputs).max(axis=2) / 127.0
    s = np.maximum(s, 1e-30).astype(np.float32)
    q = np.rint(inputs / s[:, :, None]).astype(np.int8)
    return q, s


def make_in_maps(inputs):
    q, s = _quantize(np.asarray(inputs))
    in_maps = []
    for core in range(N_CORES):
        qc = q[core * B_CORE : (core + 1) * B_CORE]
        # [pair, h, f, kb, d] -> [kb, d, pair, h, f]
        xt = qc.reshape(TOT_PAIRS, 2, F, KB, 128).transpose(3, 4, 0, 1, 2)
        in_maps.append({"xq": np.ascontiguousarray(xt)})
    return in_maps, s


def kernel(inputs: np.ndarray) -> np.ndarray:
    inputs = np.asarray(inputs)
    assert inputs.shape == (B, F, D), inputs.shape

    in_maps, s = make_in_maps(inputs)
    nc = _get_nc()
    res = run_bass_kernel_spmd(nc, in_maps, list(range(N_CORES)))

    iu, ju = np.triu_indices(F, k=1)
    outs = []
    for core in range(N_CORES):
        r = res.results[core]["out"]  # [2, F, pair, g] fp16
        gram = (
            r.transpose(2, 0, 1, 3)  # [pair, h, f, g]
            .reshape(B_CORE, F, F)
        )
        outs.append(gram[:, iu, ju].astype(np.float32))
    tri = np.concatenate(outs, axis=0)
    scale = 256.0 * s[:, iu] * s[:, ju]
    return tri * scale
